# revision 2
# baseline (speedup 1.0000x reference)
"""Autoformer-style EncoderLayer for Trainium2, data-parallel over batch
across 8 NeuronCores. Mixed-precision rewrite:

  - decomp (banded matmul) + PE transposes in bf16
  - Q/K/V projections, scores, attn@V, softmax denominator and the WO
    projection in fp8e4m3 with DoubleRow perf mode (256-deep contraction,
    2x FLOPs per instruction => ~157 TF/s)
  - FFN matmuls in bf16 (fp8 fails the precision budget)
  - biases folded: bq/bk/bb1 into psum evictions, bv/bo into a precomputed
    bo' = bv@wo + bo added to the attention residual slab, bb2 into the
    FFN2 residual slab
  - everything SBUF-resident (no DRAM spills)

Per core: one [L=2048, D=512] sequence.
"""
import math
import numpy as np
import ml_dtypes
from contextlib import ExitStack

import concourse.bass as bass
import concourse.mybir as mybir
import concourse.tile as tile
from concourse import bacc
from concourse.bass_utils import run_bass_kernel_spmd

P = 128
B_, L, D = 8, 2048, 512
KPOOL, PAD = 25, 12
EPS = 1e-5
WS = 16.0                      # fp8 scale for wq/wk/wv/wo
SCALE8 = 1.0 / (math.sqrt(D) * WS * WS)
ESHIFT = -1.5                  # softmax shift: exp(s-1.5); invariant, avoids fp8 overflow
AVS = 1.0 / 256.0              # AV psum -> fp8 scale
NLC = L // P          # 16 l-chunks of 128
NB = L // 512         # 4  l-blocks of 512
ND = D // P           # 4  d-chunks of 128

f32 = mybir.dt.float32
bf16 = mybir.dt.bfloat16
f8 = mybir.dt.float8e4
AF = mybir.ActivationFunctionType
ALU = mybir.AluOpType
DR = mybir.MatmulPerfMode.DoubleRow

_CACHE = {}


def _band_blocks():
    i = np.arange(P)[:, None]
    j = np.arange(P)[None, :]
    a = (np.abs(i - j) <= PAD).astype(np.float32) / KPOOL
    bdiag = np.eye(P, dtype=np.float32) - a
    bup = -((i - j) >= (P - PAD)).astype(np.float32) / KPOOL   # rows chunk c-1, cols chunk c
    bdown = bup.T.copy()                                       # rows chunk c+1, cols chunk c
    return bdiag, bup, bdown


def _ln_block(nc, small, t_sum, t_ssq, t_eps, n):
    """Per-block LayerNorm stats on [P, n]: returns (istd, negmean)."""
    t_mean = small.tile([P, n], f32, tag="lbm", name="tb_mean")
    nc.vector.tensor_scalar_mul(t_mean[:], t_sum[:], 1.0 / D)
    t_m2 = small.tile([P, n], f32, tag="lbm2", name="tb_m2")
    nc.vector.tensor_tensor(t_m2[:], t_mean[:], t_mean[:], ALU.mult)
    t_var = small.tile([P, n], f32, tag="lbv", name="tb_var")
    nc.vector.scalar_tensor_tensor(t_var[:], t_ssq[:], 1.0 / D, t_m2[:],
                                   op0=ALU.mult, op1=ALU.subtract)
    t_sd = small.tile([P, n], f32, tag="lbsd", name="tb_sd")
    nc.scalar.activation(t_sd[:], t_var[:], AF.Sqrt, bias=t_eps[:])
    t_istd = small.tile([P, n], f32, tag="lbi", name="tb_istd")
    nc.vector.reciprocal(t_istd[:], t_sd[:])
    t_negm = small.tile([P, n], f32, tag="lbng", name="tb_negm")
    nc.vector.tensor_scalar_mul(t_negm[:], t_mean[:], -1.0)
    return t_istd, t_negm


def _build(apply_g1, apply_g2):
    nc = bacc.Bacc("TRN2", target_bir_lowering=False, debug=False)

    x = nc.dram_tensor("x", [L, D], bf16, kind="ExternalInput").ap()
    w8 = {n: nc.dram_tensor(n, [P, ND, D], f8, kind="ExternalInput").ap()
          for n in ["wq8", "wk8", "wv8", "wo8"]}
    wb = {n: nc.dram_tensor(n, [P, ND, D], bf16, kind="ExternalInput").ap()
          for n in ["w1b", "w2b"]}
    ncb = 1536 + (1024 if apply_g1 else 0) + (1024 if apply_g2 else 0)
    cb16 = nc.dram_tensor("cb16", [P, ncb], bf16, kind="ExternalInput").ap()
    cf32 = nc.dram_tensor("cf32", [P, 16], f32, kind="ExternalInput").ap()

    out = nc.dram_tensor("out", [L, D], f32, kind="ExternalOutput").ap()
    out_c = out.rearrange("(l p) d -> l p d", p=P)

    with tile.TileContext(nc) as tc, ExitStack() as ctx:
        misc = ctx.enter_context(tc.tile_pool(name="misc", bufs=1))
        small = ctx.enter_context(tc.tile_pool(name="small", bufs=4))

        # ---- constants ----
        t_cb = misc.tile([P, ncb], bf16, name="t_cb")
        nc.sync.dma_start(t_cb[:], cb16)
        t_cf = misc.tile([P, 16], f32, name="t_cf")
        nc.sync.dma_start(t_cf[:], cf32)
        t_bd = t_cb[:, 0:128]
        t_bu = t_cb[:, 128:256]
        t_bn = t_cb[:, 256:384]
        t_id = t_cb[:, 384:512]
        t_bob = t_cb[:, 512:1024]     # bo' = bv@wo + bo, broadcast [P, 512]
        t_bb2 = t_cb[:, 1024:1536]    # bb2 broadcast [P, 512]
        off = 1536
        t_gb = {}
        if apply_g1:
            t_gb["g1"] = t_cb[:, off:off + 512]
            t_gb["be1"] = t_cb[:, off + 512:off + 1024]
            off += 1024
        if apply_g2:
            t_gb["g2"] = t_cb[:, off:off + 512]
            t_gb["be2"] = t_cb[:, off + 512:off + 1024]
            off += 1024
        t_bqs = t_cf[:, 0:4]          # 16*bq, [P, dc]
        t_bks = t_cf[:, 4:8]
        t_b1s = t_cf[:, 8:12]
        t_eps = t_cf[:, 12:13]
        t_o2 = t_cf[0:1, 13:15]       # [1, 2] ones, f32
        t_esh = t_cf[:, 15:16]        # exp shift (-1.5)
        t_ones8 = misc.tile([P, 2, 128], f8, name="t_ones8")
        nc.vector.memset(t_ones8[:], 1.0)

        # ---- weights ----
        t_w8 = {}
        for n in ["wq8", "wk8", "wv8", "wo8"]:
            t_w8[n] = misc.tile([P, ND, D], f8, name=f"t_{n}")
            nc.sync.dma_start(t_w8[n][:], w8[n])
        t_wb = {}
        for n in ["w1b", "w2b"]:
            t_wb[n] = misc.tile([P, ND, D], bf16, name=f"t_{n}")
            nc.sync.dma_start(t_wb[n][:], wb[n])

        # ---- stats tiles ----
        t_sum1 = misc.tile([P, NLC], f32, name="t_sum1")
        t_ssq1 = misc.tile([P, NLC], f32, name="t_ssq1")
        t_sum2 = misc.tile([P, NLC], f32, name="t_sum2")
        t_ssq2 = misc.tile([P, NLC], f32, name="t_ssq2")
        t_rec = misc.tile([P, NLC], f32, name="t_rec")
        t_den = misc.tile([1, L], f32, name="t_den")

        # ---- persistent activations ----
        t_sr = misc.tile([P, NLC, D], bf16, name="t_sr")      # s + bo'
        t_tr = misc.tile([P, NLC, D], bf16, name="t_tr")      # trend

        x_cview = x.rearrange("(l p) d -> p l d", p=P)

        es_qkv = ExitStack()   # st/qt/kt: freed after scores (LIFO: opened last)
        es_uv = ExitStack()    # v/u/avt/ffn pools: persist to the end
        es_avt = ExitStack()
        es_ffn = ExitStack()
        try:
            uvp = es_uv.enter_context(tc.tile_pool(name="uvp", bufs=1))
            t_v = uvp.tile([P, NLC, D], f8, name="t_v")
            t_u = uvp.tile([P, NLC, L], f8, name="t_u")
            qkvp = es_qkv.enter_context(tc.tile_pool(name="qkvp", bufs=1))
            t_st = qkvp.tile([P, ND, L], f8, name="t_st")
            t_qt = qkvp.tile([P, ND, L], f8, name="t_qt")
            t_kt = qkvp.tile([P, ND, L], f8, name="t_kt")

            # ================= Phase 1: decomp + S^T =================
            with tc.tile_pool(name="xin", bufs=6) as xin, \
                 tc.tile_pool(name="sstage", bufs=3) as sstage, \
                 tc.tile_pool(name="ps_d", bufs=3, space="PSUM") as ps_d, \
                 tc.tile_pool(name="ps_tr", bufs=4, space="PSUM") as ps_tr:
                x_ch = {}

                def get_x(j):
                    if j not in x_ch:
                        t = xin.tile([P, D], bf16, tag="xw", name=f"xw{j}")
                        nc.sync.dma_start(t[:], x_cview[:, j, :])
                        x_ch[j] = t
                    return x_ch[j]

                for j in range(4):
                    get_x(j)

                for lc in range(NLC):
                    pss = ps_d.tile([P, D], f32, tag="dmm", name="pss")
                    nbrs = [(lc - 1, t_bu), (lc, t_bd), (lc + 1, t_bn)]
                    nbrs = [(j, t) for j, t in nbrs if 0 <= j < NLC]
                    for i, (j, tb) in enumerate(nbrs):
                        nc.tensor.matmul(pss[:], tb, get_x(j)[:],
                                         start=(i == 0), stop=(i == len(nbrs) - 1))
                    t_stmp = sstage.tile([P, D], bf16, tag="st", name="t_stmp")
                    if lc % 2 == 0:
                        nc.scalar.copy(t_stmp[:], pss[:])
                    else:
                        nc.vector.tensor_copy(t_stmp[:], pss[:])
                    nc.vector.scalar_tensor_tensor(t_sr[:, lc, :], pss[:], 1.0,
                                                   t_bob, op0=ALU.mult, op1=ALU.add)
                    nc.gpsimd.tensor_tensor(t_tr[:, lc, :], get_x(lc)[:],
                                            t_stmp[:], ALU.subtract)
                    ptr = ps_tr.tile([P, ND, P], bf16, tag="tr", name="ptr")
                    for dc in range(ND):
                        nc.tensor.transpose(ptr[:, dc, :],
                                            t_stmp[:, bass.ts(dc, P)], t_id)
                    if lc % 2 == 0:
                        nc.vector.tensor_copy(t_st[:, :, bass.ts(lc, P)], ptr[:])
                    else:
                        nc.scalar.copy(t_st[:, :, bass.ts(lc, P)], ptr[:])

            # ================= Phase 2: projections (fp8 DR) =================
            # Order: KT, V, QT — so scores can begin right after QT, and the
            # Scalar engine is free of V evictions during the exp-heavy phase.
            with tc.tile_pool(name="ps_p", bufs=8, space="PSUM") as ps_p:
                ei = 0
                for tdst, wname, bcol in [(t_kt, "wk8", t_bks), (None, "wv8", None),
                                          (t_qt, "wq8", t_bqs)]:
                    if tdst is None:
                        # V: [m, d] natural, lhsT = S^T slices (bias folded)
                        for mc in range(NLC):
                            pv = ps_p.tile([P, D], f32, tag="mm", name="pv")
                            for kp in range(2):
                                nc.tensor.matmul(
                                    pv[:], t_st[:, 2 * kp:2 * kp + 2, bass.ts(mc, P)],
                                    t_w8["wv8"][:, 2 * kp:2 * kp + 2, :],
                                    start=(kp == 0), stop=(kp == 1), perf_mode=DR)
                            if ei % 2 == 0:
                                nc.scalar.copy(t_v[:, mc, :], pv[:])
                            else:
                                nc.vector.tensor_copy(t_v[:, mc, :], pv[:])
                            ei += 1
                        continue
                    for half in range(2):
                        pq = {}
                        for kp in range(2):
                            for dc in range(ND):
                                for li, lb in enumerate((2 * half, 2 * half + 1)):
                                    key = (dc, li)
                                    if kp == 0:
                                        pq[key] = ps_p.tile([P, 512], f32, tag="mm",
                                                            name=f"pq{dc}_{li}")
                                    nc.tensor.matmul(
                                        pq[key][:],
                                        t_w8[wname][:, 2 * kp:2 * kp + 2, bass.ts(dc, P)],
                                        t_st[:, 2 * kp:2 * kp + 2, bass.ts(lb, 512)],
                                        start=(kp == 0), stop=(kp == 1), perf_mode=DR)
                        for dc in range(ND):
                            for li, lb in enumerate((2 * half, 2 * half + 1)):
                                if ei % 2 == 0:
                                    nc.scalar.activation(
                                        tdst[:, dc, bass.ts(lb, 512)], pq[(dc, li)][:],
                                        AF.Identity, bias=bcol[:, dc:dc + 1])
                                else:
                                    nc.vector.tensor_scalar(
                                        tdst[:, dc, bass.ts(lb, 512)], pq[(dc, li)][:],
                                        bcol[:, dc:dc + 1], None, op0=ALU.add)
                                ei += 1

            # ================= Phase 3: scores + exp =================
            with tc.tile_pool(name="ps_s", bufs=8, space="PSUM") as ps_s:
                for mc in range(NLC):
                    psc = {}
                    for kp in range(2):
                        for lb in range(NB):
                            if kp == 0:
                                psc[lb] = ps_s.tile([P, 512], f32, tag="sc",
                                                    name=f"psc{lb}")
                            nc.tensor.matmul(
                                psc[lb][:],
                                t_kt[:, 2 * kp:2 * kp + 2, bass.ts(mc, P)],
                                t_qt[:, 2 * kp:2 * kp + 2, bass.ts(lb, 512)],
                                start=(kp == 0), stop=(kp == 1), perf_mode=DR)
                    for lb in range(NB):
                        nc.scalar.activation(t_u[:, mc, bass.ts(lb, 512)],
                                             psc[lb][:], AF.Exp, scale=SCALE8,
                                             bias=t_esh)
            es_qkv.close()

            # ================= Phase 4: dens + rec =================
            with tc.tile_pool(name="ps_de", bufs=4, space="PSUM") as ps_de, \
                 tc.tile_pool(name="ps_rc", bufs=1, space="PSUM") as ps_rc:
                pden = {}
                for mcp in range(8):
                    for lb in range(NB):
                        if mcp == 0:
                            pden[lb] = ps_de.tile([1, 512], f32, tag="den",
                                                  name=f"pden{lb}")
                        nc.tensor.matmul(
                            pden[lb][:], t_ones8[:, :, 0:1],
                            t_u[:, 2 * mcp:2 * mcp + 2, bass.ts(lb, 512)],
                            start=(mcp == 0), stop=(mcp == 7), perf_mode=DR)
                for lb in range(NB):
                    nc.scalar.copy(t_den[:, bass.ts(lb, 512)], pden[lb][:])
                prc = ps_rc.tile([P, NLC, 2], f32, tag="rc", name="prc")
                for c in range(NLC):
                    nc.tensor.matmul(prc[:, c, :], t_den[:, bass.ts(c, P)],
                                     t_o2, start=True, stop=True)
                nc.vector.reciprocal(t_rec[:], prc[:, :, 0])

            # ================= Phase 5: AV (fp8 DR) -> avt8 =================
            avtp = es_avt.enter_context(tc.tile_pool(name="avtp", bufs=1))
            t_avt = avtp.tile([P, ND, L], f8, name="t_avt")
            with tc.tile_pool(name="ps_av", bufs=8, space="PSUM") as ps_av:
                ei = 0
                for half in range(2):
                    pav = {}
                    for mcp in range(8):
                        for dc in range(ND):
                            for li, lb in enumerate((2 * half, 2 * half + 1)):
                                key = (dc, li)
                                if mcp == 0:
                                    pav[key] = ps_av.tile([P, 512], f32, tag="av",
                                                          name=f"pav{dc}_{li}")
                                nc.tensor.matmul(
                                    pav[key][:],
                                    t_v[:, 2 * mcp:2 * mcp + 2, bass.ts(dc, P)],
                                    t_u[:, 2 * mcp:2 * mcp + 2, bass.ts(lb, 512)],
                                    start=(mcp == 0), stop=(mcp == 7), perf_mode=DR)
                    for dc in range(ND):
                        for li, lb in enumerate((2 * half, 2 * half + 1)):
                            if ei % 2 == 0:
                                nc.scalar.activation(t_avt[:, dc, bass.ts(lb, 512)],
                                                     pav[(dc, li)][:], AF.Copy,
                                                     scale=AVS)
                            else:
                                nc.vector.tensor_scalar(
                                    t_avt[:, dc, bass.ts(lb, 512)],
                                    pav[(dc, li)][:], AVS, None, op0=ALU.mult)
                            ei += 1

            # ================= Phase 6: WO (fp8 DR) + LN1 + h^T =================
            ffnp = es_ffn.enter_context(tc.tile_pool(name="ffnp", bufs=1))
            t_h = ffnp.tile([P, NLC, D], bf16, name="t_h")
            t_hb = ffnp.tile([P, NLC, D], bf16, name="t_hb")
            t_ht = ffnp.tile([P, ND, L], bf16, name="t_ht")
            with tc.tile_pool(name="ps_wo", bufs=4, space="PSUM") as ps_wo, \
                 tc.tile_pool(name="ps_t2", bufs=4, space="PSUM") as ps_t2, \
                 tc.tile_pool(name="rsst", bufs=6) as rsst:
                for lb in range(NB):
                    rs_list = []
                    for c in range(4):
                        lc = lb * 4 + c
                        pwo = ps_wo.tile([P, D], f32, tag="wo", name="pwo")
                        for kp in range(2):
                            nc.tensor.matmul(
                                pwo[:], t_avt[:, 2 * kp:2 * kp + 2, bass.ts(lc, P)],
                                t_w8["wo8"][:, 2 * kp:2 * kp + 2, :],
                                start=(kp == 0), stop=(kp == 1), perf_mode=DR)
                        t_rs = rsst.tile([P, D], bf16, tag="rs", name="t_rs")
                        nc.vector.scalar_tensor_tensor(
                            t_rs[:], pwo[:], t_rec[:, lc:lc + 1], t_sr[:, lc, :],
                            op0=ALU.mult, op1=ALU.add,
                            accum_out=t_sum1[:, lc:lc + 1])
                        t_scr = rsst.tile([P, D], bf16, tag="sq", name="t_scr")
                        nc.scalar.activation(t_scr[:], t_rs[:], AF.Square,
                                             accum_out=t_ssq1[:, lc:lc + 1])
                        rs_list.append(t_rs)
                    t_istd4, t_negm4 = _ln_block(
                        nc, small, t_sum1[:, lb * 4:lb * 4 + 4],
                        t_ssq1[:, lb * 4:lb * 4 + 4], t_eps, 4)
                    for c in range(4):
                        lc = lb * 4 + c
                        nc.vector.tensor_scalar(t_h[:, lc, :], rs_list[c][:],
                                                t_negm4[:, c:c + 1],
                                                t_istd4[:, c:c + 1],
                                                op0=ALU.add, op1=ALU.mult)
                        if apply_g1:
                            nc.vector.tensor_tensor(t_h[:, lc, :], t_h[:, lc, :],
                                                    t_gb["g1"], ALU.mult)
                            nc.vector.tensor_tensor(t_h[:, lc, :], t_h[:, lc, :],
                                                    t_gb["be1"], ALU.add)
                        nc.gpsimd.tensor_tensor(t_hb[:, lc, :], t_h[:, lc, :],
                                                t_bb2, ALU.add)
                        ptr = ps_t2.tile([P, ND, P], bf16, tag="tr2", name="ptr2")
                        for dc in range(ND):
                            nc.tensor.transpose(ptr[:, dc, :],
                                                t_h[:, lc, bass.ts(dc, P)], t_id)
                        if c % 2 == 0:
                            nc.scalar.copy(t_ht[:, :, bass.ts(lc, P)], ptr[:])
                        else:
                            nc.vector.tensor_copy(t_ht[:, :, bass.ts(lc, P)], ptr[:])

            # ================= Phase 7: FFN1 (bf16) =================
            t_rt = ffnp.tile([P, ND, L], bf16, name="t_rt")
            with tc.tile_pool(name="ps_f1", bufs=8, space="PSUM") as ps_f1:
                ei = 0
                for half in range(2):
                    pf = {}
                    for k in range(ND):
                        for dc in range(ND):
                            for li, lb in enumerate((2 * half, 2 * half + 1)):
                                key = (dc, li)
                                if k == 0:
                                    pf[key] = ps_f1.tile([P, 512], f32, tag="f1",
                                                         name=f"pf{dc}_{li}")
                                nc.tensor.matmul(
                                    pf[key][:],
                                    t_wb["w1b"][:, k, bass.ts(dc, P)],
                                    t_ht[:, k, bass.ts(lb, 512)],
                                    start=(k == 0), stop=(k == ND - 1))
                    for dc in range(ND):
                        for li, lb in enumerate((2 * half, 2 * half + 1)):
                            if ei % 2 == 0:
                                nc.scalar.activation(
                                    t_rt[:, dc, bass.ts(lb, 512)], pf[(dc, li)][:],
                                    AF.Relu, bias=t_b1s[:, dc:dc + 1])
                            else:
                                nc.vector.tensor_scalar(
                                    t_rt[:, dc, bass.ts(lb, 512)], pf[(dc, li)][:],
                                    t_b1s[:, dc:dc + 1], 0.0,
                                    op0=ALU.add, op1=ALU.max)
                            ei += 1

            # ================= Phase 8: FFN2 + LN2 + out =================
            with tc.tile_pool(name="ps_f2", bufs=4, space="PSUM") as ps_f2, \
                 tc.tile_pool(name="fst", bufs=6) as fst, \
                 tc.tile_pool(name="ost", bufs=4) as ost:
                for lb in range(NB):
                    res_list = []
                    for c in range(4):
                        lc = lb * 4 + c
                        pf2 = ps_f2.tile([P, D], f32, tag="f2", name="pf2")
                        for k in range(ND):
                            nc.tensor.matmul(pf2[:], t_rt[:, k, bass.ts(lc, P)],
                                             t_wb["w2b"][:, k, :],
                                             start=(k == 0), stop=(k == ND - 1))
                        t_res = fst.tile([P, D], bf16, tag="res", name="t_res")
                        nc.vector.scalar_tensor_tensor(
                            t_res[:], pf2[:], 1.0, t_hb[:, lc, :],
                            op0=ALU.mult, op1=ALU.add,
                            accum_out=t_sum2[:, lc:lc + 1])
                        t_scr = fst.tile([P, D], bf16, tag="sq2", name="t_scr2")
                        nc.scalar.activation(t_scr[:], t_res[:], AF.Square,
                                             accum_out=t_ssq2[:, lc:lc + 1])
                        res_list.append(t_res)
                    t_istd4, t_negm4 = _ln_block(
                        nc, small, t_sum2[:, lb * 4:lb * 4 + 4],
                        t_ssq2[:, lb * 4:lb * 4 + 4], t_eps, 4)
                    for c in range(4):
                        lc = lb * 4 + c
                        t_h2 = fst.tile([P, D], bf16, tag="h2", name="t_h2")
                        nc.vector.tensor_scalar(t_h2[:], res_list[c][:],
                                                t_negm4[:, c:c + 1],
                                                t_istd4[:, c:c + 1],
                                                op0=ALU.add, op1=ALU.mult)
                        if apply_g2:
                            nc.vector.tensor_tensor(t_h2[:], t_h2[:],
                                                    t_gb["g2"], ALU.mult)
                            nc.vector.tensor_tensor(t_h2[:], t_h2[:],
                                                    t_gb["be2"], ALU.add)
                        t_out = ost.tile([P, D], f32, tag="o", name="t_out")
                        eng = nc.gpsimd if c % 2 == 0 else nc.vector
                        eng.tensor_tensor(t_out[:], t_h2[:], t_tr[:, lc, :],
                                          ALU.add)
                        nc.sync.dma_start(out_c[lc], t_out[:])
        finally:
            es_ffn.close()
            es_avt.close()
            es_qkv.close()
            es_uv.close()

    nc.compile()
    return nc


def _consts(inp, apply_g1, apply_g2):
    bdiag, bup, bdown = _band_blocks()
    ncb = 1536 + (1024 if apply_g1 else 0) + (1024 if apply_g2 else 0)
    cb = np.zeros((P, ncb), np.float32)
    cb[:, 0:128] = bdiag
    cb[:, 128:256] = bup
    cb[:, 256:384] = bdown
    cb[:, 384:512] = np.eye(P, dtype=np.float32)
    bo_p = inp["bv"] @ inp["wo"] + inp["bo"]
    cb[:, 512:1024] = bo_p.reshape(1, D)
    cb[:, 1024:1536] = inp["bb2"].reshape(1, D)
    off = 1536
    if apply_g1:
        cb[:, off:off + 512] = inp["g1"].reshape(1, D)
        cb[:, off + 512:off + 1024] = inp["be1"].reshape(1, D)
        off += 1024
    if apply_g2:
        cb[:, off:off + 512] = inp["g2"].reshape(1, D)
        cb[:, off + 512:off + 1024] = inp["be2"].reshape(1, D)
        off += 1024
    cf = np.zeros((P, 16), np.float32)
    cf[:, 0:4] = (inp["bq"] * WS).reshape(ND, P).T
    cf[:, 4:8] = (inp["bk"] * WS).reshape(ND, P).T
    cf[:, 8:12] = inp["bb1"].reshape(ND, P).T
    cf[:, 12] = EPS
    cf[:, 13:15] = 1.0
    cf[:, 15] = ESHIFT

    def pack_w(w):
        return np.ascontiguousarray(w.reshape(ND, P, D).transpose(1, 0, 2))

    consts = {
        "wq8": (pack_w(inp["wq"]) * WS).astype(ml_dtypes.float8_e4m3fn),
        "wk8": (pack_w(inp["wk"]) * WS).astype(ml_dtypes.float8_e4m3fn),
        "wv8": (pack_w(inp["wv"]) * WS).astype(ml_dtypes.float8_e4m3fn),
        "wo8": (pack_w(inp["wo"]) * WS).astype(ml_dtypes.float8_e4m3fn),
        "w1b": pack_w(inp["w1"]).astype(ml_dtypes.bfloat16),
        "w2b": pack_w(inp["w2"]).astype(ml_dtypes.bfloat16),
        "cb16": cb.astype(ml_dtypes.bfloat16),
        "cf32": cf,
    }
    return consts


def _prepare(inputs):
    inp = {k: np.ascontiguousarray(np.asarray(v, dtype=np.float32))
           for k, v in inputs.items()}
    x = inp["x"]                      # [8, 2048, 512]
    assert x.shape == (B_, L, D)

    apply_g1 = not (np.allclose(inp["g1"], 1.0) and np.allclose(inp["be1"], 0.0))
    apply_g2 = not (np.allclose(inp["g2"], 1.0) and np.allclose(inp["be2"], 0.0))

    key = (apply_g1, apply_g2)
    if key not in _CACHE:
        _CACHE[key] = _build(apply_g1, apply_g2)
    nc = _CACHE[key]

    consts = _consts(inp, apply_g1, apply_g2)
    x8 = x.astype(ml_dtypes.bfloat16)
    in_maps = [dict(consts, x=np.ascontiguousarray(x8[i])) for i in range(B_)]
    return nc, in_maps


def kernel(**inputs):
    nc, in_maps = _prepare(inputs)
    res = run_bass_kernel_spmd(nc, in_maps, core_ids=list(range(B_)))
    return np.stack([res.results[i]["out"] for i in range(B_)], axis=0)
